# revision 1
# baseline (speedup 1.0000x reference)
"""DLSTMCell hypernetwork kernel for Trainium2 (8 NeuronCores).

Reference computation (per stock n of N=2048):
    mem  = emb_table[index]                       (N, 128)
    h1   = tanh(mem @ w1.T + b1)                  (N, 128)
    h    = tanh(h1 @ w2.T + b2)                   (N, 64)
    W_n  = (h @ w3.T + b3).reshape(192, 512)      per-stock LSTM weights
    z    = data_n @ W_n + lstm_bias               data = [x, hx]  (192,)
    g    = sigmoid(z); i,f,gg,o = split(g)
    cy   = cx*sigmoid(f) + sigmoid(i)*tanh(gg)
    hy   = sigmoid(o)*tanh(cy)

Key fusion: the (N, 192, 512) = 805MB weights tensor is never materialized.
    z[n,k] = sum_{d,b} (data[n,d]*h[n,b]) * W3perm[(d,b),k] + sum_d B3r[d,k]*data[n,d]
is a standard dense matmul with the SHARED (12288, 512) matrix W3perm against
per-stock outer-product tiles opT[(d,b), n], accumulated in PSUM.

Sharding: data-parallel over stocks on 8 cores (256 each). Microbenched
(loop-slope) per-MM rates: accumulation-chain MMs run at stream rate with
LDWEIGHTS hidden (~130-150ns at N=256); multi-core throttling at N=256 is
only ~4-6%, so 8x256 beats 4x512 (which throttles ~+43%). W3perm is
replicated per core but stored fp8e4 (x64 scale, descaled in the gate
sigmoid) to halve the replicated HBM traffic; fp8-stationary x bf16-moving
measured fastest at this shape. No DoubleRow: measured 603ns per DR-MM vs
2x203ns for the bf16 pair it replaces.

Stage A (op-tile construction) uses SELECTOR matmuls: pa2[p, n] =
data[n, 2t+(p>=64)] is built by one K=64/128 matmul per K-tile with a
constant 0/1 fp8 selector as stationary and the bf16 data tile as moving
(full-partition, offset-0, small-pitch moving reads measured ~131ns/MM).
The previous dPair3 packing (2-partition DMA slices at bases {0,32,64} +
base-rotated moving reads) hit a hardware-only failure at NC_N=256 and
slow big-pitch moving reads; selectors avoid both. Then
opT = pa2 * [hT; hT] on the vector engine, two K-tiles per DVE op.

Precision: w3 fp8e4 (x64), selectors fp8 (exact 0/1), data/op bf16,
MLP f32r, PSUM always f32. End-to-end rel err ~1e-4 vs the 2e-2 budget
(the hypernetwork matmul term is small against lstm_bias and two sigmoids
compress errors).

Layout: gates transposed [k, n] so gate unit k sits on partitions:
lstm_bias folds into the ACT sigmoid as a per-partition bias and the LSTM
epilogue runs on [128, n] tiles. Gate PSUM tiles are padded to full 2KB
banks: half-bank tiles can share a bank and concurrent PE-write + ACT-read
of two tiles in one bank is a fatal HW PSUM collision.
"""
import sys

sys.path.insert(0, "/opt/trn_rl_repo")
import numpy as np
import ml_dtypes
import concourse.bacc as bacc
import concourse.mybir as mybir
import concourse.tile as tile
from concourse.bass_utils import run_bass_kernel_spmd

F32 = mybir.dt.float32
F32R = mybir.dt.float32r
BF16 = mybir.dt.bfloat16
FP8 = mybir.dt.float8e4
AF = mybir.ActivationFunctionType

W8 = True                   # w3 stationary in fp8e4 (plain matmul, no DoubleRow)
W3_SCALE = 64.0             # keeps w3 (~0.02 sigma) in e4m3 normal range
Z_DESCALE = 1.0 / W3_SCALE

N = 2048
INPUT = 64
EMB = 128
BOT = 64
HID = 128
WDIM = 4 * (INPUT + HID) * HID
NCORES = 8
NC_N = N // NCORES          # 256 stocks per core
D = INPUT + HID             # 192
K4 = 4 * HID                # 512 gate columns
KT = (D * BOT) // 128       # 96 contraction K-tiles of 128
KU = KT // 2                # 48 paired K-tiles
NCHUNK = 8                  # w3 chunk DMAs per iteration
NCH_U = KU // NCHUNK        # pair-units per chunk
T0 = INPUT // 2             # K-tiles 0..31 draw data rows from x (d < 64)

W3DT = FP8 if W8 else BF16

_cache = {}


def _build_program(repeat=1):
    """repeat>1 wraps the compute body in a hardware loop — used only for
    wall-clock slope timing (exec_ns ~= (wall[R2]-wall[R1])/(R2-R1))."""
    nc = bacc.Bacc(None)

    di = lambda name, shape, dt: nc.dram_tensor(name, shape, dt, kind="ExternalInput")
    memT_d = di("memT", [EMB, NC_N], F32R)
    dT0_d = di("dT0b", [INPUT, NC_N], BF16)
    dT1_d = di("dT1b", [HID, NC_N], BF16)
    cxT_d = di("cxT", [HID, NC_N], F32)
    w1T_d = di("w1T", [EMB, EMB], F32R)
    w2T_d = di("w2T", [EMB, BOT], F32R)
    b1_d = di("b1c", [EMB, 1], F32)
    b2_d = di("b2c", [BOT, 1], F32)
    # W3perm flat: partition p holds every pair-unit's row p, so chunk DMAs
    # move NCH_U KB per partition-line (descriptor-efficient ~400GB/s vs
    # ~95GB/s for 1KB lines). w3flat[p, u*1024 + h*512 + k] =
    # W3perm[u*128+p, ...] for K-tiles (2u | 2u+1).
    w3p_d = di("w3p", [128, KU * 2 * K4], W3DT)
    b3a_d = di("b3a", [INPUT, K4], BF16)
    b3b_d = di("b3b", [HID, K4], BF16)
    lb_d = di("lbias", [HID, 4], F32)
    # selectors: sel0[k, t*128+p] = (k == 2t + (p>=64)) for K-tiles 0..31
    # (data rows in x), sel1 likewise for K-tiles 32..95 (rows in hx).
    sel0_d = di("sel0", [INPUT, T0 * 128], FP8)
    sel1_d = di("sel1", [HID, (KT - T0) * 128], FP8)
    hyT_o = nc.dram_tensor("hyT", [HID, NC_N], F32, kind="ExternalOutput")
    cyT_o = nc.dram_tensor("cyT", [HID, NC_N], F32, kind="ExternalOutput")

    with tile.TileContext(nc) as tc:
        with tc.tile_pool(name="const", bufs=1) as const, \
             tc.tile_pool(name="w3", bufs=1) as w3pool, \
             tc.tile_pool(name="op", bufs=4) as opool, \
             tc.tile_pool(name="ep", bufs=1) as ep, \
             tc.tile_pool(name="psA", bufs=2, space="PSUM") as psA, \
             tc.tile_pool(name="psG", bufs=1, space="PSUM") as psG:

            def load(dram, shape, dt, tag=None):
                nm = tag or dram.name
                t = const.tile(shape, dt, tag=nm, name=nm)
                nc.sync.dma_start(t[:], dram[:])
                return t

            memT = load(memT_d, [EMB, NC_N], F32R)
            dT0b = load(dT0_d, [INPUT, NC_N], BF16)
            dT1b = load(dT1_d, [HID, NC_N], BF16)
            cxT = load(cxT_d, [HID, NC_N], F32)
            w1T = load(w1T_d, [EMB, EMB], F32R)
            w2T = load(w2T_d, [EMB, BOT], F32R)
            b1c = load(b1_d, [EMB, 1], F32)
            b2c = load(b2_d, [BOT, 1], F32)
            b3a = load(b3a_d, [INPUT, K4], BF16)
            b3b = load(b3b_d, [HID, K4], BF16)
            lb = load(lb_d, [HID, 4], F32)
            sel0 = load(sel0_d, [INPUT, T0 * 128], FP8)
            sel1 = load(sel1_d, [HID, (KT - T0) * 128], FP8)

            from contextlib import ExitStack
            loop_ctx = ExitStack()
            if repeat > 1:
                loop_ctx.enter_context(
                    tc.For_i(0, repeat, 1, hint_engines=(mybir.EngineType.PE,))
                )

            # gate accumulators [k-chunk, n], one full PSUM bank each
            psg = [
                psG.tile([128, 512], F32, tag=f"g{kc}", name=f"psg{kc}")[:, 0:NC_N]
                for kc in range(4)
            ]

            # hypernetwork MLP (PSUM scratch borrowed from psg banks; the
            # later start=True bias matmuls reset them for accumulation)
            nc.tensor.matmul(psg[0][:], w1T[:], memT[:], start=True, stop=True)
            h1T = ep.tile([128, NC_N], F32R, tag="h1T")
            nc.scalar.activation(h1T[:], psg[0][:], AF.Tanh, bias=b1c[:])
            nc.tensor.matmul(psg[1][0:BOT, :], w2T[:], h1T[:], start=True, stop=True)
            hT2 = ep.tile([128, NC_N], F32R, tag="hT2")
            nc.scalar.activation(hT2[0:BOT, :], psg[1][0:BOT, :], AF.Tanh, bias=b2c[:])
            nc.scalar.activation(hT2[BOT:128, :], psg[1][0:BOT, :], AF.Tanh, bias=b2c[:])

            # fold the b3 term in first (start=True resets the banks)
            for kc in range(4):
                ks = slice(kc * 128, kc * 128 + 128)
                nc.tensor.matmul(psg[kc][:], b3a[:, ks], dT0b[:], start=True, stop=False)
                nc.tensor.matmul(psg[kc][:], b3b[:, ks], dT1b[:], start=False, stop=False)

            # main contraction: 48 pair-units u = K-tiles (2u, 2u+1).
            # Stage A (per u): two selector matmuls -> pa2 [128, 2*NC_N] PSUM,
            # one DVE mul -> op2 [128, 2, NC_N] bf16, one w3 DMA.
            # Gate matmuls consume pair u LA units later.
            LA = 2
            op_q = []

            # w3 chunk prefetch: NCHUNK big DMAs, each its own tile so unit
            # u's gate matmuls wait only on chunk u//NCH_U.
            w3ch = []
            for i in range(NCHUNK):
                wt = w3pool.tile([128, NCH_U * 2 * K4], W3DT, tag=f"w3c{i}",
                                 name=f"w3c{i}")
                nc.sync.dma_start(
                    wt[:], w3p_d[:, i * NCH_U * 2 * K4:(i + 1) * NCH_U * 2 * K4])
                w3ch.append(wt)

            def w3slice(u, h, kc):
                base = (u % NCH_U) * 2 * K4 + h * K4 + kc * 128
                return w3ch[u // NCH_U][:, base:base + 128]

            def emit_stage_a(u):
                pa2 = psA.tile([128, 2 * NC_N], F32, tag="A", name="pa2")
                for h in range(2):
                    t = 2 * u + h
                    if t < T0:
                        sel, mv = sel0[:, t * 128:(t + 1) * 128], dT0b
                    else:
                        sel = sel1[:, (t - T0) * 128:(t - T0 + 1) * 128]
                        mv = dT1b
                    nc.tensor.matmul(
                        pa2[:, h * NC_N:(h + 1) * NC_N], sel, mv[:],
                        start=True, stop=True,
                    )
                op2 = opool.tile([128, 2, NC_N], BF16, tag="opT", name="op2")
                nc.vector.tensor_mul(
                    op2[:],
                    pa2[:].rearrange("p (h n) -> p h n", h=2),
                    hT2[:, None, :].broadcast_to([128, 2, NC_N]),
                )
                op_q.append(op2)

            for u in range(min(LA, KU)):
                emit_stage_a(u)
            for u in range(KU):
                if u + LA < KU:
                    emit_stage_a(u + LA)
                last = u == KU - 1
                for h in range(2):
                    for kc in range(4):
                        nc.tensor.matmul(
                            psg[kc][:],
                            w3slice(u, h, kc),
                            op_q[u][:, h, :],
                            start=False, stop=last and h == 1,
                        )
                op_q[u] = None

            # LSTM epilogue on [hid, n] tiles; k-chunk order: i, f, g, o
            g = []
            for kc in range(4):
                gt = ep.tile([128, NC_N], F32, tag=f"gs{kc}", name=f"gs{kc}")
                nc.scalar.activation(gt[:], psg[kc][:], AF.Sigmoid,
                                     bias=lb[:, kc:kc + 1],
                                     scale=Z_DESCALE if W8 else 1.0)
                g.append(gt)
            i_t = ep.tile([128, NC_N], F32, tag="i_t")
            nc.scalar.activation(i_t[:], g[0][:], AF.Sigmoid)
            f_t = ep.tile([128, NC_N], F32, tag="f_t")
            nc.scalar.activation(f_t[:], g[1][:], AF.Sigmoid)
            g_t = ep.tile([128, NC_N], F32, tag="g_t")
            nc.scalar.activation(g_t[:], g[2][:], AF.Tanh)
            o_t = ep.tile([128, NC_N], F32, tag="o_t")
            nc.scalar.activation(o_t[:], g[3][:], AF.Sigmoid)

            t1 = ep.tile([128, NC_N], F32, tag="t1")
            nc.vector.tensor_mul(t1[:], cxT[:], f_t[:])
            t2 = ep.tile([128, NC_N], F32, tag="t2")
            nc.vector.tensor_mul(t2[:], i_t[:], g_t[:])
            cy = ep.tile([128, NC_N], F32, tag="cy")
            nc.vector.tensor_add(cy[:], t1[:], t2[:])
            tcy = ep.tile([128, NC_N], F32, tag="tcy")
            nc.scalar.activation(tcy[:], cy[:], AF.Tanh)
            hy = ep.tile([128, NC_N], F32, tag="hy")
            nc.vector.tensor_mul(hy[:], o_t[:], tcy[:])

            nc.sync.dma_start(cyT_o[:], cy[:])
            nc.sync.dma_start(hyT_o[:], hy[:])

            loop_ctx.close()

    nc.finalize()
    return nc


def _consts():
    """Input-independent constant tensors (selectors)."""
    if "sel" not in _cache:
        sel0 = np.zeros((INPUT, T0 * 128), np.float32)
        for t in range(T0):
            sel0[2 * t, t * 128:t * 128 + 64] = 1.0
            sel0[2 * t + 1, t * 128 + 64:t * 128 + 128] = 1.0
        sel1 = np.zeros((HID, (KT - T0) * 128), np.float32)
        for i in range(KT - T0):
            sel1[2 * i, i * 128:i * 128 + 64] = 1.0
            sel1[2 * i + 1, i * 128 + 64:i * 128 + 128] = 1.0
        _cache["sel"] = (sel0.astype(ml_dtypes.float8_e4m3),
                         sel1.astype(ml_dtypes.float8_e4m3))
    return _cache["sel"]


def kernel(x, index, hx, cx, emb_table, w1, b1, w2, b2, w3, b3, lstm_bias,
           _trace=False):
    x = np.asarray(x, np.float32)
    index = np.asarray(index)
    hx = np.asarray(hx, np.float32)
    cx = np.asarray(cx, np.float32)
    emb_table = np.asarray(emb_table, np.float32)
    w1 = np.asarray(w1, np.float32)
    b1 = np.asarray(b1, np.float32)
    w2 = np.asarray(w2, np.float32)
    b2 = np.asarray(b2, np.float32)
    w3 = np.asarray(w3, np.float32)
    b3 = np.asarray(b3, np.float32)
    lstm_bias = np.asarray(lstm_bias, np.float32)

    if "nc" not in _cache:
        _cache["nc"] = _build_program()
    nc = _cache["nc"]
    sel0, sel1 = _consts()

    # host-side input prep (sharding + layout)
    mem = emb_table[index]                                   # (N, EMB)
    c = np.ascontiguousarray
    w1T = c(w1.T)
    w2T = c(w2.T)
    b1c = b1.reshape(EMB, 1)
    b2c = b2.reshape(BOT, 1)
    # W3perm[(d*64+b), k] = w3[d*512+k, b]; then pair K-tiles (2u, 2u+1)
    w3perm = w3.reshape(D, K4, BOT).transpose(0, 2, 1).reshape(D * BOT, K4)
    w3pair = w3perm.reshape(KU, 2, 128, K4).transpose(0, 2, 1, 3)
    # flat: [128, KU * 2 * K4], partition p holds all units' row p
    w3flat = w3pair.transpose(1, 0, 2, 3).reshape(128, KU * 2 * K4)
    if W8:
        w3p = c(w3flat * W3_SCALE).astype(ml_dtypes.float8_e4m3)
        zs = W3_SCALE
    else:
        w3p = c(w3flat).astype(ml_dtypes.bfloat16)
        zs = 1.0
    b3r = b3.reshape(D, K4) * zs
    b3a = c(b3r[0:INPUT]).astype(ml_dtypes.bfloat16)
    b3b = c(b3r[INPUT:D]).astype(ml_dtypes.bfloat16)
    lbias = c(lstm_bias.reshape(4, HID).T)                   # [j, kc]

    xT = x.T.astype(ml_dtypes.bfloat16)                      # (64, N)
    hxT = hx.T.astype(ml_dtypes.bfloat16)                    # (128, N)
    memTf = mem.T                                            # (128, N)
    cxTf = cx.T                                              # (128, N)

    in_maps = []
    for ci in range(NCORES):
        sl = slice(ci * NC_N, (ci + 1) * NC_N)
        in_maps.append({
            "memT": c(memTf[:, sl]),
            "dT0b": c(xT[:, sl]),
            "dT1b": c(hxT[:, sl]),
            "cxT": c(cxTf[:, sl]),
            "w1T": w1T, "w2T": w2T, "b1c": b1c, "b2c": b2c,
            "w3p": w3p, "b3a": b3a, "b3b": b3b,
            "lbias": lbias, "sel0": sel0, "sel1": sel1,
        })

    res = run_bass_kernel_spmd(nc, in_maps, list(range(NCORES)), trace=_trace)
    hy = np.concatenate([r["hyT"].T for r in res.results], axis=0)
    cy = np.concatenate([r["cyT"].T for r in res.results], axis=0)
    if _trace:
        kernel.last_results = res
    return hy.astype(np.float32), cy.astype(np.float32)

